# revision 15
# baseline (speedup 1.0000x reference)
"""Trainium2 Bass kernel for CustomGRU (B=64, T=512, D=512, U=1024).

Sharding: data-parallel over batch across 8 NeuronCores (8 rows each),
weights replicated. Everything runs U-major ("flipped" dataflow):

  out[u_chunk(128), b] = sum_k U_chunk[k,:].T @ hT_chunk[k]  (+ x-proj, bias)

i.e. the 128x128 weight chunk is the PE stationary and the [128, bss]
h^T chunk is the moving operand, so each matmul streams only bss moving
rows (vs 512 when weights are the moving side). Gates come out of PSUM
already transposed ([u, b]); sigmoid/tanh/combine all run on U-major
tiles, so no PE transposes and no partition-block reduction are needed
anywhere in the recurrence.

The per-core batch (8 rows) is further split into NS independent
recurrence streams of bss = 8/NS rows. The streams' serial dependency
chains (r-mm -> sigmoid -> r*h -> hh-mm -> tanh -> combine) interleave
on the engines, hiding the fixed cross-engine latencies (~200ns/hop)
that otherwise dominate a single chain.

Per stream per step t (all fp16 except PSUM):
  - z/r/h pre-acts: per u-chunk [128,bss] PSUM region: bias-mm (K=1,
    start=True) + 4 x-projection mms (stationary = W d-chunk, moving =
    resident x^T slice) + 8 U-mms (moving = h^T chunk). The bias/x mms
    of step t+1 are emitted during step t's tail, where the PE would
    otherwise idle waiting on tanh/combine (they don't depend on h).
  - z and r accumulate in separate PSUM tiles (a shared tile serializes
    sigma(r) against the z-matmul writes at tile granularity).
  - sigmoid(r) -> rh = r*h (DVE) -> candidate U-mms -> tanh ->
    h = hh*(1-z) + z*h_prev, with (1-z) and z*h_prev precomputed while
    the candidate matmuls run, so only mul+add remain after tanh.
  - h history accumulates U-major in a [128, 8*KC*bss] tile; one DMA
    out per stream per 8 steps.

x is fully SBUF-resident ([128, 4*BS*T] fp16, 32KB/partition), so the
recurrence does no input DMA at all.
"""
import sys

if "/opt/trn_rl_repo" not in sys.path:
    sys.path.insert(0, "/opt/trn_rl_repo")

import numpy as np
from contextlib import ExitStack

import concourse.bass as bass
import concourse.bacc as bacc
import concourse.tile as tile
from concourse import mybir
from concourse.bass_utils import run_bass_kernel_spmd

F32 = mybir.dt.float32
F16 = mybir.dt.float16
SIG = mybir.ActivationFunctionType.Sigmoid
TANH = mybir.ActivationFunctionType.Tanh

N_CORES = 8
B = 64
BS = B // N_CORES  # 8 batch rows per core
D = 512
U = 1024
U3 = 3 * U         # z|r|h
KC = U // 128      # 8 contraction chunks over hidden
DC = D // 128      # 4 contraction chunks over input dim
OUT_BLK = 8        # h steps buffered per output DMA
NS = 1             # independent batch streams per core
BSS = BS // NS     # batch rows per stream
SL = KC * BSS      # h-state columns per stream per step


def build(nc, T, reps=1):
    assert T % OUT_BLK == 0
    BT = BS * T
    NB = T // OUT_BLK

    # ---- DRAM I/O (per-core) ----
    xT_d = nc.dram_tensor("xT", [D, BT], F16, kind="ExternalInput")
    w_d = nc.dram_tensor("wcat", [D, U3], F16, kind="ExternalInput")
    brow_d = nc.dram_tensor("brow", [1, U3], F16, kind="ExternalInput")
    uzr_d = nc.dram_tensor("uzr", [U, 2 * U], F16, kind="ExternalInput")
    uh_d = nc.dram_tensor("uh", [U, U], F16, kind="ExternalInput")
    out_d = nc.dram_tensor("out", [NB, 128, OUT_BLK * KC * BS], F16,
                           kind="ExternalOutput")

    with tile.TileContext(nc) as tc, ExitStack() as ctx:
        const = ctx.enter_context(tc.tile_pool(name="const", bufs=1))
        wp = ctx.enter_context(tc.tile_pool(name="w", bufs=1))

        # resident weights / x, all [p, chunk, cols] fp16
        x_sb = wp.tile([128, DC * BT], F16)
        nc.sync.dma_start(
            x_sb[:].rearrange("p (dc bt) -> p dc bt", dc=DC),
            xT_d.rearrange("(dc p) bt -> p dc bt", p=128),
        )
        w_sb = wp.tile([128, DC * U3], F16)
        nc.sync.dma_start(
            w_sb[:].rearrange("p (dc u) -> p dc u", dc=DC),
            w_d.rearrange("(dc p) u -> p dc u", p=128),
        )
        uzr_sb = wp.tile([128, KC * 2 * U], F16)
        nc.sync.dma_start(
            uzr_sb[:].rearrange("p (k u) -> p k u", k=KC),
            uzr_d.rearrange("(k p) u -> p k u", p=128),
        )
        uh_sb = wp.tile([128, KC * U], F16)
        nc.sync.dma_start(
            uh_sb[:].rearrange("p (k u) -> p k u", k=KC),
            uh_d.rearrange("(k p) u -> p k u", p=128),
        )
        brow = const.tile([1, U3], F16)
        nc.sync.dma_start(brow[:], brow_d[:])
        ones1 = const.tile([1, BSS], F16)
        nc.any.memset(ones1[:], 1.0)
        zerohs = []
        for s in range(NS):
            zh0 = const.tile([128, SL], F16, name=f"zeroh{s}")
            nc.any.memzero(zh0[:])
            zerohs.append(zh0)

        hist_p = ctx.enter_context(tc.tile_pool(name="hist", bufs=3))
        gates = ctx.enter_context(tc.tile_pool(name="gates", bufs=2))
        psum_bufs = 2 if NS == 1 else 1   # 8 PSUM banks total, 1 bank/tile
        pz_p = ctx.enter_context(
            tc.tile_pool(name="pz", bufs=psum_bufs, space="PSUM"))
        pr_p = ctx.enter_context(
            tc.tile_pool(name="pr", bufs=psum_bufs, space="PSUM"))
        ph_p = ctx.enter_context(
            tc.tile_pool(name="ph", bufs=psum_bufs, space="PSUM"))

        def biasx(ps, cu0, s, t):
            """bias + x-projection start mms for 8 u-chunks (global chunk
            index cu0..cu0+8) of stream s, step t; h-independent."""
            for i in range(8):
                cu = cu0 + i
                reg = ps[:, i * BSS:(i + 1) * BSS]
                nc.tensor.matmul(reg, brow[:, cu * 128:(cu + 1) * 128],
                                 ones1[:], start=True, stop=False,
                                 skip_group_check=True)
                xoff = t * BS + s * BSS
                for dc in range(DC):
                    nc.tensor.matmul(
                        reg,
                        w_sb[:, dc * U3 + cu * 128: dc * U3 + (cu + 1) * 128],
                        x_sb[:, dc * BT + xoff: dc * BT + xoff + BSS],
                        start=False, stop=False, skip_group_check=True)

        def u_mms(ps, usb, ustride, uoff, mov_tile, mov_off):
            """8 u-chunks x 8 k-chunks of hidden-to-hidden matmuls."""
            for c in range(8):
                reg = ps[:, c * BSS:(c + 1) * BSS]
                for k in range(KC):
                    nc.tensor.matmul(
                        ps[:, c * BSS:(c + 1) * BSS],
                        usb[:, k * ustride + uoff + c * 128:
                            k * ustride + uoff + (c + 1) * 128],
                        mov_tile[:, mov_off + k * BSS: mov_off + (k + 1) * BSS],
                        start=False, stop=(k == KC - 1),
                        skip_group_check=True)

        # per-stream rotating state
        st = []
        for s in range(NS):
            pz = pz_p.tile([128, 8 * BSS], F32, tag=f"pz{s}")
            pr = pr_p.tile([128, 8 * BSS], F32, tag=f"pr{s}")
            ph = ph_p.tile([128, 8 * BSS], F32, tag=f"ph{s}")
            biasx(pz, 0, s, 0)
            biasx(pr, 8, s, 0)
            biasx(ph, 16, s, 0)
            st.append({"pz": pz, "pr": pr, "ph": ph,
                       "hpt": zerohs[s], "hpo": 0, "hist": None})

        for rep in range(reps):
          for t in range(T):
            slot = t % OUT_BLK
            last = (t == T - 1) and (rep == reps - 1)
            prev_rh = None
            for s in range(NS):
                ss = st[s]
                if t == 0 and rep == 0 and s > 0 and prev_rh is not None:
                    # phase-stagger stream s behind stream s-1: rewrite its
                    # zero initial state (still all-zeros) with a dependency
                    # on the previous stream's first r*h product, so the
                    # streams' serial chains interleave in anti-phase on the
                    # engines instead of running in lock-step.
                    nc.vector.tensor_scalar_mul(zerohs[s][:], prev_rh[:], 0.0)
                if slot == 0:
                    ss["hist"] = hist_p.tile([128, OUT_BLK * SL], F16,
                                             tag=f"hist{s}", name=f"hist{s}")
                hist, hpt, hpo = ss["hist"], ss["hpt"], ss["hpo"]
                pz, pr, ph = ss["pz"], ss["pr"], ss["ph"]
                if not last:
                    pz_n = pz_p.tile([128, 8 * BSS], F32, tag=f"pz{s}")
                    pr_n = pr_p.tile([128, 8 * BSS], F32, tag=f"pr{s}")
                    ph_n = ph_p.tile([128, 8 * BSS], F32, tag=f"ph{s}")

                # r gate, then sigmoid + r*h on the side
                u_mms(pr, uzr_sb, 2 * U, U, hpt, hpo)
                rT = gates.tile([128, SL], F16, tag=f"rT{s}")
                nc.scalar.activation(rT[:], pr[:], SIG)
                rh = gates.tile([128, SL], F16, tag=f"rh{s}")
                nc.vector.tensor_mul(rh[:], rT[:], hpt[:, hpo:hpo + SL])
                if t == 0 and rep == 0:
                    prev_rh = rh

                # z gate; precompute zb = 1-z and zh = z*h_prev off the
                # critical path (so only mul+add remain after tanh)
                u_mms(pz, uzr_sb, 2 * U, 0, hpt, hpo)
                zT = gates.tile([128, SL], F16, tag=f"zT{s}")
                nc.scalar.activation(zT[:], pz[:], SIG)
                zb = gates.tile([128, SL], F16, tag=f"zb{s}")
                nc.gpsimd.tensor_scalar(zb[:], zT[:], -1.0, 1.0,
                                        mybir.AluOpType.mult,
                                        mybir.AluOpType.add)
                zh = gates.tile([128, SL], F16, tag=f"zh{s}")
                nc.gpsimd.tensor_mul(zh[:], zT[:], hpt[:, hpo:hpo + SL])

                # PE tail filler: next step's h-independent start mms
                if not last:
                    biasx(pz_n, 0, s, t + 1)
                    biasx(pr_n, 8, s, t + 1)

                # candidate
                u_mms(ph, uh_sb, U, 0, rh, 0)
                if not last:
                    biasx(ph_n, 16, s, t + 1)
                hh = gates.tile([128, SL], F16, tag=f"hh{s}")
                nc.scalar.activation(hh[:], ph[:], TANH)

                # combine: h = hh*(1-z) + z*h_prev, into the history slot
                tmp = gates.tile([128, SL], F16, tag=f"tmp{s}")
                nc.vector.tensor_mul(tmp[:], hh[:], zb[:])
                nc.vector.tensor_add(hist[:, slot * SL:(slot + 1) * SL],
                                     tmp[:], zh[:])

                if slot == OUT_BLK - 1:
                    nc.sync.dma_start(
                        out_d[t // OUT_BLK].squeeze()[
                            :, s * OUT_BLK * SL:(s + 1) * OUT_BLK * SL],
                        hist[:])

                ss["hpt"], ss["hpo"] = hist, slot * SL
                if not last:
                    ss["pz"], ss["pr"], ss["ph"] = pz_n, pr_n, ph_n

    nc.compile()
    return nc


def prepare(inputs, Wz, Uz, bz, Wr, Ur, br, Wh, Uh, bh, T):
    """Build the Bass program and the per-core input maps."""
    x = np.asarray(inputs, dtype=np.float32)[:, :T, :]

    wcat = np.concatenate([Wz, Wr, Wh], axis=1).astype(np.float16)
    brow = np.concatenate([bz, br, bh]).astype(np.float16)[None, :]
    uzr = np.concatenate([Uz, Ur], axis=1).astype(np.float16)
    uh = np.asarray(Uh).astype(np.float16)

    nc = bacc.Bacc("TRN2", target_bir_lowering=False, debug=False,
                   num_devices=N_CORES)
    build(nc, T)

    in_maps = []
    for c in range(N_CORES):
        xc = x[c * BS:(c + 1) * BS]                    # [BS, T, D]
        # x^T, t-major columns: xT[d, t*BS + b] = xc[b, t, d]
        xT = np.ascontiguousarray(
            xc.transpose(2, 1, 0).reshape(D, T * BS)).astype(np.float16)
        in_maps.append({
            "xT": xT, "wcat": wcat, "brow": brow, "uzr": uzr, "uh": uh,
        })
    return nc, in_maps


def assemble(results):
    T = results[0]["out"].shape[0] * OUT_BLK
    outs = []
    for c in range(N_CORES):
        o = results[c]["out"]                # [NB, 128, NS*OUT_BLK*KC*BSS]
        o = o.reshape(T // OUT_BLK, 128, NS, OUT_BLK, KC, BSS)
        # [blk, p, s, slot, c8, j] -> [s, j, blk, slot, c8, p] -> [BS, T, U]
        h = o.transpose(2, 5, 0, 3, 4, 1).reshape(BS, T, U)
        outs.append(h.astype(np.float32))
    return np.concatenate(outs, axis=0)                # [B, T, U]


def kernel(inputs, Wz, Uz, bz, Wr, Ur, br, Wh, Uh, bh, _T=None):
    T = inputs.shape[1] if _T is None else _T
    nc, in_maps = prepare(inputs, Wz, Uz, bz, Wr, Ur, br, Wh, Uh, bh, T)
    res = run_bass_kernel_spmd(nc, in_maps, list(range(N_CORES)))
    return assemble(res.results)


# revision 16
# speedup vs baseline: 1.0541x; 1.0541x over previous
"""Trainium2 Bass kernel for CustomGRU (B=64, T=512, D=512, U=1024).

Sharding: data-parallel over batch across 8 NeuronCores (8 rows each),
weights replicated. Everything runs U-major ("flipped" dataflow):

  out[u_chunk(128), b] = sum_k U_chunk[k,:].T @ hT_chunk[k]  (+ x-proj, bias)

i.e. the 128x128 weight chunk is the PE stationary and the [128, bss]
h^T chunk is the moving operand, so each matmul streams only bss moving
rows (vs 512 when weights are the moving side). Gates come out of PSUM
already transposed ([u, b]); sigmoid/tanh/combine all run on U-major
tiles, so no PE transposes and no partition-block reduction are needed
anywhere in the recurrence.

The per-core batch (8 rows) is further split into NS independent
recurrence streams of bss = 8/NS rows. The streams' serial dependency
chains (r-mm -> sigmoid -> r*h -> hh-mm -> tanh -> combine) interleave
on the engines, hiding the fixed cross-engine latencies (~200ns/hop)
that otherwise dominate a single chain.

Per stream per step t (all fp16 except PSUM):
  - z/r/h pre-acts: per u-chunk [128,bss] PSUM region: bias-mm (K=1,
    start=True) + 4 x-projection mms (stationary = W d-chunk, moving =
    resident x^T slice) + 8 U-mms (moving = h^T chunk). The bias/x mms
    of step t+1 are emitted during step t's tail, where the PE would
    otherwise idle waiting on tanh/combine (they don't depend on h).
  - z and r accumulate in separate PSUM tiles (a shared tile serializes
    sigma(r) against the z-matmul writes at tile granularity).
  - sigmoid(r) -> rh = r*h (DVE) -> candidate U-mms -> tanh ->
    h = hh*(1-z) + z*h_prev, with (1-z) and z*h_prev precomputed while
    the candidate matmuls run, so only mul+add remain after tanh.
  - h history accumulates U-major in a [128, 8*KC*bss] tile; one DMA
    out per stream per 8 steps.

x is fully SBUF-resident ([128, 4*BS*T] fp16, 32KB/partition), so the
recurrence does no input DMA at all.
"""
import sys

if "/opt/trn_rl_repo" not in sys.path:
    sys.path.insert(0, "/opt/trn_rl_repo")

import numpy as np
from contextlib import ExitStack

import concourse.bass as bass
import concourse.bacc as bacc
import concourse.tile as tile
from concourse import mybir
from concourse.bass_utils import run_bass_kernel_spmd

F32 = mybir.dt.float32
F16 = mybir.dt.float16
SIG = mybir.ActivationFunctionType.Sigmoid
TANH = mybir.ActivationFunctionType.Tanh

N_CORES = 8
B = 64
BS = B // N_CORES  # 8 batch rows per core
D = 512
U = 1024
U3 = 3 * U         # z|r|h
KC = U // 128      # 8 contraction chunks over hidden
DC = D // 128      # 4 contraction chunks over input dim
OUT_BLK = 8        # h steps buffered per output DMA
NS = 1             # independent batch streams per core
BSS = BS // NS     # batch rows per stream
SL = KC * BSS      # h-state columns per stream per step


def build(nc, T, reps=1):
    assert T % OUT_BLK == 0
    BT = BS * T
    NB = T // OUT_BLK

    # ---- DRAM I/O (per-core) ----
    xT_d = nc.dram_tensor("xT", [D, BT], F16, kind="ExternalInput")
    w_d = nc.dram_tensor("wcat", [D, U3], F16, kind="ExternalInput")
    brow_d = nc.dram_tensor("brow", [1, U3], F16, kind="ExternalInput")
    uzr_d = nc.dram_tensor("uzr", [U, 2 * U], F16, kind="ExternalInput")
    uh_d = nc.dram_tensor("uh", [U, U], F16, kind="ExternalInput")
    out_d = nc.dram_tensor("out", [NB, 128, OUT_BLK * KC * BS], F16,
                           kind="ExternalOutput")

    with tile.TileContext(nc) as tc, ExitStack() as ctx:
        const = ctx.enter_context(tc.tile_pool(name="const", bufs=1))
        wp = ctx.enter_context(tc.tile_pool(name="w", bufs=1))

        # resident weights / x, all [p, chunk, cols] fp16
        x_sb = wp.tile([128, DC * BT], F16)
        nc.sync.dma_start(
            x_sb[:].rearrange("p (dc bt) -> p dc bt", dc=DC),
            xT_d.rearrange("(dc p) bt -> p dc bt", p=128),
        )
        w_sb = wp.tile([128, DC * U3], F16)
        nc.sync.dma_start(
            w_sb[:].rearrange("p (dc u) -> p dc u", dc=DC),
            w_d.rearrange("(dc p) u -> p dc u", p=128),
        )
        uzr_sb = wp.tile([128, KC * 2 * U], F16)
        nc.sync.dma_start(
            uzr_sb[:].rearrange("p (k u) -> p k u", k=KC),
            uzr_d.rearrange("(k p) u -> p k u", p=128),
        )
        uh_sb = wp.tile([128, KC * U], F16)
        nc.sync.dma_start(
            uh_sb[:].rearrange("p (k u) -> p k u", k=KC),
            uh_d.rearrange("(k p) u -> p k u", p=128),
        )
        brow = const.tile([1, U3], F16)
        nc.sync.dma_start(brow[:], brow_d[:])
        ones1 = const.tile([1, BSS], F16)
        nc.any.memset(ones1[:], 1.0)
        zerohs = []
        for s in range(NS):
            zh0 = const.tile([128, SL], F16, name=f"zeroh{s}")
            nc.any.memzero(zh0[:])
            zerohs.append(zh0)

        hist_p = ctx.enter_context(tc.tile_pool(name="hist", bufs=3))
        gates = ctx.enter_context(tc.tile_pool(name="gates", bufs=2))
        psum_bufs = 2 if NS == 1 else 1   # 8 PSUM banks total, 1 bank/tile
        pz_p = ctx.enter_context(
            tc.tile_pool(name="pz", bufs=psum_bufs, space="PSUM"))
        pr_p = ctx.enter_context(
            tc.tile_pool(name="pr", bufs=psum_bufs, space="PSUM"))
        ph_p = ctx.enter_context(
            tc.tile_pool(name="ph", bufs=psum_bufs, space="PSUM"))

        def biasx(ps, cu0, s, t):
            """bias + x-projection start mms for 8 u-chunks (global chunk
            index cu0..cu0+8) of stream s, step t; h-independent."""
            for i in range(8):
                cu = cu0 + i
                reg = ps[:, i * BSS:(i + 1) * BSS]
                nc.tensor.matmul(reg, brow[:, cu * 128:(cu + 1) * 128],
                                 ones1[:], start=True, stop=False,
                                 skip_group_check=True)
                xoff = t * BS + s * BSS
                for dc in range(DC):
                    nc.tensor.matmul(
                        reg,
                        w_sb[:, dc * U3 + cu * 128: dc * U3 + (cu + 1) * 128],
                        x_sb[:, dc * BT + xoff: dc * BT + xoff + BSS],
                        start=False, stop=False, skip_group_check=True)

        def u_mms(ps, usb, ustride, uoff, mov_tile, mov_off):
            """8 u-chunks x 8 k-chunks of hidden-to-hidden matmuls."""
            for c in range(8):
                reg = ps[:, c * BSS:(c + 1) * BSS]
                for k in range(KC):
                    nc.tensor.matmul(
                        ps[:, c * BSS:(c + 1) * BSS],
                        usb[:, k * ustride + uoff + c * 128:
                            k * ustride + uoff + (c + 1) * 128],
                        mov_tile[:, mov_off + k * BSS: mov_off + (k + 1) * BSS],
                        start=False, stop=(k == KC - 1),
                        skip_group_check=True)

        # per-stream rotating state
        st = []
        for s in range(NS):
            pz = pz_p.tile([128, 8 * BSS], F32, tag=f"pz{s}")
            pr = pr_p.tile([128, 8 * BSS], F32, tag=f"pr{s}")
            ph = ph_p.tile([128, 8 * BSS], F32, tag=f"ph{s}")
            biasx(pz, 0, s, 0)
            biasx(pr, 8, s, 0)
            biasx(ph, 16, s, 0)
            st.append({"pz": pz, "pr": pr, "ph": ph,
                       "hpt": zerohs[s], "hpo": 0, "hist": None})

        for rep in range(reps):
          for t in range(T):
            slot = t % OUT_BLK
            last = (t == T - 1) and (rep == reps - 1)
            prev_rh = None
            for s in range(NS):
                ss = st[s]
                if t == 0 and rep == 0 and s > 0 and prev_rh is not None:
                    # phase-stagger stream s behind stream s-1: rewrite its
                    # zero initial state (still all-zeros) with a dependency
                    # on the previous stream's first r*h product, so the
                    # streams' serial chains interleave in anti-phase on the
                    # engines instead of running in lock-step.
                    nc.vector.tensor_scalar_mul(zerohs[s][:], prev_rh[:], 0.0)
                if slot == 0:
                    ss["hist"] = hist_p.tile([128, OUT_BLK * SL], F16,
                                             tag=f"hist{s}", name=f"hist{s}")
                hist, hpt, hpo = ss["hist"], ss["hpt"], ss["hpo"]
                pz, pr, ph = ss["pz"], ss["pr"], ss["ph"]
                if not last:
                    pz_n = pz_p.tile([128, 8 * BSS], F32, tag=f"pz{s}")
                    pr_n = pr_p.tile([128, 8 * BSS], F32, tag=f"pr{s}")
                    ph_n = ph_p.tile([128, 8 * BSS], F32, tag=f"ph{s}")

                # r gate, then sigmoid + r*h on the side
                u_mms(pr, uzr_sb, 2 * U, U, hpt, hpo)
                rT = gates.tile([128, SL], F16, tag=f"rT{s}")
                nc.scalar.activation(rT[:], pr[:], SIG)
                rh = gates.tile([128, SL], F16, tag=f"rh{s}")
                nc.vector.tensor_mul(rh[:], rT[:], hpt[:, hpo:hpo + SL])
                if t == 0 and rep == 0:
                    prev_rh = rh

                # z gate; precompute zb = 1-z and zh = z*h_prev off the
                # critical path (so only mul+add remain after tanh)
                u_mms(pz, uzr_sb, 2 * U, 0, hpt, hpo)
                zT = gates.tile([128, SL], F16, tag=f"zT{s}")
                nc.scalar.activation(zT[:], pz[:], SIG)
                zb = gates.tile([128, SL], F16, tag=f"zb{s}")
                nc.vector.tensor_scalar(zb[:], zT[:], -1.0, 1.0,
                                        mybir.AluOpType.mult,
                                        mybir.AluOpType.add)
                zh = gates.tile([128, SL], F16, tag=f"zh{s}")
                nc.vector.tensor_mul(zh[:], zT[:], hpt[:, hpo:hpo + SL])

                # PE tail filler: next step's h-independent start mms
                if not last:
                    biasx(pz_n, 0, s, t + 1)
                    biasx(pr_n, 8, s, t + 1)

                # candidate
                u_mms(ph, uh_sb, U, 0, rh, 0)
                if not last:
                    biasx(ph_n, 16, s, t + 1)
                hh = gates.tile([128, SL], F16, tag=f"hh{s}")
                nc.scalar.activation(hh[:], ph[:], TANH)

                # combine: h = hh*(1-z) + z*h_prev, into the history slot
                tmp = gates.tile([128, SL], F16, tag=f"tmp{s}")
                nc.vector.tensor_mul(tmp[:], hh[:], zb[:])
                nc.vector.tensor_add(hist[:, slot * SL:(slot + 1) * SL],
                                     tmp[:], zh[:])

                if slot == OUT_BLK - 1:
                    nc.sync.dma_start(
                        out_d[t // OUT_BLK].squeeze()[
                            :, s * OUT_BLK * SL:(s + 1) * OUT_BLK * SL],
                        hist[:])

                ss["hpt"], ss["hpo"] = hist, slot * SL
                if not last:
                    ss["pz"], ss["pr"], ss["ph"] = pz_n, pr_n, ph_n

    nc.compile()
    return nc


def prepare(inputs, Wz, Uz, bz, Wr, Ur, br, Wh, Uh, bh, T):
    """Build the Bass program and the per-core input maps."""
    x = np.asarray(inputs, dtype=np.float32)[:, :T, :]

    wcat = np.concatenate([Wz, Wr, Wh], axis=1).astype(np.float16)
    brow = np.concatenate([bz, br, bh]).astype(np.float16)[None, :]
    uzr = np.concatenate([Uz, Ur], axis=1).astype(np.float16)
    uh = np.asarray(Uh).astype(np.float16)

    nc = bacc.Bacc("TRN2", target_bir_lowering=False, debug=False,
                   num_devices=N_CORES)
    build(nc, T)

    in_maps = []
    for c in range(N_CORES):
        xc = x[c * BS:(c + 1) * BS]                    # [BS, T, D]
        # x^T, t-major columns: xT[d, t*BS + b] = xc[b, t, d]
        xT = np.ascontiguousarray(
            xc.transpose(2, 1, 0).reshape(D, T * BS)).astype(np.float16)
        in_maps.append({
            "xT": xT, "wcat": wcat, "brow": brow, "uzr": uzr, "uh": uh,
        })
    return nc, in_maps


def assemble(results):
    T = results[0]["out"].shape[0] * OUT_BLK
    outs = []
    for c in range(N_CORES):
        o = results[c]["out"]                # [NB, 128, NS*OUT_BLK*KC*BSS]
        o = o.reshape(T // OUT_BLK, 128, NS, OUT_BLK, KC, BSS)
        # [blk, p, s, slot, c8, j] -> [s, j, blk, slot, c8, p] -> [BS, T, U]
        h = o.transpose(2, 5, 0, 3, 4, 1).reshape(BS, T, U)
        outs.append(h.astype(np.float32))
    return np.concatenate(outs, axis=0)                # [B, T, U]


def kernel(inputs, Wz, Uz, bz, Wr, Ur, br, Wh, Uh, bh, _T=None):
    T = inputs.shape[1] if _T is None else _T
    nc, in_maps = prepare(inputs, Wz, Uz, bz, Wr, Ur, br, Wh, Uh, bh, T)
    res = run_bass_kernel_spmd(nc, in_maps, list(range(N_CORES)))
    return assemble(res.results)
